# revision 14
# baseline (speedup 1.0000x reference)
"""Trainium2 Bass kernel for DCTEncoderLayer — v3.2 "stationary swap, batched DMA".

Separable 32x32 DCT with the YCbCr mix folded into stage-1 weights.
Stage 1 runs with the IMAGE as the PE stationary operand:

    t1T[x, (c,v)] = img_chunk[(c',y), x].T @ W1[(c',y), (c,v)]

which lands the stage-1 result already transposed (x on partitions) —
no DVE stream-transpose is needed anywhere.  Stage 2 is a single
128-partition block-diagonal DCT along x':

    out[(gxl,u), (kk,c,v)] = W2bd[(gxl,x'), (gxl,u)].T @ t1s[(gxl,x'), (kk,c,v)]

v3.2 vs v3.1: DMAs carry ~350ns fixed cost each, so input DMAs batch 4
block-rows (host lays x out so each partition reads 4KB contiguously)
and output DMAs batch 4 iterations' results; stage-2 PSUM pairs two
iterations per 2-bank tile so each cast2 covers 768 columns.  Output
DMAs issue from the otherwise idle GpSimd DGE queue, input from Sync.

Per block-row iteration (64 per core, grouped by 4):
  - 4 stage-1 matmuls: stationary img chunk [96,128] fp16, moving W1 [96,96]
  - cast1 (ACT/DVE alt): t1T PSUM f32 [128, 4x(96 of 128)] -> SBUF fp16 [128,384]
  - 1 stage-2 matmul: stationary W2bd [128,128] fp16, moving [128,384] fp16
  - per 2 iters: cast2 (ACT/DVE alt) o2p PSUM f32 -> fp16 into the group's
    staging tile; per 4 iters: one output DMA [128, 1536] fp16.
Host upcasts, permutes axes and applies the frequency sort.
"""

import os
import sys

try:
    import concourse.bass  # noqa: F401
except ImportError:
    sys.path.insert(0, "/opt/trn_rl_repo")

import numpy as np

import concourse.bacc as bacc
import concourse.bass as bass
import concourse.mybir as mybir
import concourse.tile as tile
from concourse.bass_utils import run_bass_kernel_spmd

F32 = mybir.dt.float32
F16 = mybir.dt.float16

BS = 32
N_CORES = 8
B_PER_CORE = 4
NH = 16
ITERS = B_PER_CORE * NH  # 64 block-rows per core
GROUPS = ITERS // 4      # 16 groups of 4 block-rows

_STATE = {}
LAST_RESULT = None


def _dct_mat():
    y = np.arange(BS)
    v = np.arange(BS)[:, None]
    c = np.cos((2 * y + 1) * v * np.pi / (2 * BS))
    c[0, :] *= 1.0 / np.sqrt(2.0)
    return c / 4.0


def _sort_idx():
    mag = np.zeros((BS, BS), dtype=np.float64)
    for v in range(BS):
        for u in range(BS):
            mag[v, u] = np.linalg.norm(np.array([v, u], dtype=np.int64))
    return np.argsort(mag.reshape(-1))


def _constants():
    cs = _dct_mat()
    a2 = np.array(
        [
            [2 * 0.299, 2 * 0.587, 2 * 0.114],
            [2 * 0.564 * -0.299, 2 * 0.564 * -0.587, 2 * 0.564 * (1 - 0.114)],
            [2 * 0.713 * (1 - 0.299), 2 * 0.713 * -0.587, 2 * 0.713 * -0.114],
        ],
        np.float64,
    )
    w1 = np.zeros((96, 96))  # [(c', y), (c, v)]
    for cp in range(3):
        for c in range(3):
            w1[cp * 32 : (cp + 1) * 32, c * 32 : (c + 1) * 32] = a2[c, cp] * cs.T
    w2 = np.zeros((128, 128))  # [(gxl, x'), (gxl, u)] block diagonal over gxl
    for g in range(4):
        w2[g * 32 : (g + 1) * 32, g * 32 : (g + 1) * 32] = cs.T
    return w1.astype(np.float16), w2.astype(np.float16)


def _build_program():
    nc = bacc.Bacc(trn_type="TRN2")
    # host pre-groups 4 block-rows so each partition's 4KB is contiguous
    x = nc.dram_tensor("x", [GROUPS, 96, 4, 512], F16, kind="ExternalInput")
    w1 = nc.dram_tensor("w1", [96, 96], F16, kind="ExternalInput")
    w2 = nc.dram_tensor("w2", [128, 128], F16, kind="ExternalInput")
    out = nc.dram_tensor("out", [GROUPS, 128, 1536], F16, kind="ExternalOutput")

    with tile.TileContext(nc) as tc:
        with (
            tc.tile_pool(name="const", bufs=1) as constp,
            tc.tile_pool(name="pin", bufs=5) as pin,
            tc.tile_pool(name="pmid", bufs=8) as pmid,
            tc.tile_pool(name="pout", bufs=4) as pout,
            tc.tile_pool(name="psA", bufs=2, space="PSUM") as psA,
            tc.tile_pool(name="psB", bufs=2, space="PSUM") as psB,
        ):
            w1s = constp.tile([96, 96], F16)
            w2s = constp.tile([128, 128], F16)
            nc.sync.dma_start(w1s[:], w1[:])
            nc.sync.dma_start(w2s[:], w2[:])

            for g in range(GROUPS):
                img4 = pin.tile([96, 2048], F16, tag="img4")
                nc.sync.dma_start(
                    img4[:].rearrange("p (r x) -> p r x", r=4), x[g]
                )
                osb4 = pout.tile([128, 1536], F16, tag="osb4")
                for half in range(2):
                    hidx = 2 * g + half
                    # stage 1 (stationary swap) for 2 block-rows: 8 chunk
                    # slots of 128 f32 across a 2-bank tile
                    t1T = psA.tile([128, 1024], F32, tag="t1T")
                    for j2 in range(2):
                        j = half * 2 + j2
                        for k in range(4):
                            s = j2 * 4 + k
                            nc.tensor.matmul(
                                t1T[:, s * 128 : s * 128 + 96],
                                img4[:, j * 512 + k * 128 : j * 512 + (k + 1) * 128],
                                w1s[:],
                                start=True,
                                stop=True,
                            )
                    # cast1: pack 8 chunk slots -> contiguous fp16 [128, 768]
                    t1s = pmid.tile([128, 768], F16, tag="t1s")
                    src = t1T[:].rearrange("p (k s) -> p k s", k=8)[:, :, 0:96]
                    dst = t1s[:].rearrange("p (k s) -> p k s", k=8)
                    # cast1 lives on ACT only: alternating engines per half
                    # queues cast1(h) behind cast2(h-1) on the same engine,
                    # serializing the PE->cast1->PE->cast2 chain
                    nc.scalar.copy(dst, src)
                    # stage 2: one matmul per block-row into a shared 2-bank tile
                    o2p = psB.tile([128, 1024], F32, tag="o2p")
                    for j2 in range(2):
                        nc.tensor.matmul(
                            o2p[:, j2 * 512 : j2 * 512 + 384],
                            w2s[:],
                            t1s[:, j2 * 384 : (j2 + 1) * 384],
                            start=True,
                            stop=True,
                        )
                    # cast2 for both block-rows at once
                    csrc = o2p[:].rearrange("p (r s) -> p r s", r=2)[:, :, 0:384]
                    cdst = osb4[:, half * 768 : (half + 1) * 768].rearrange(
                        "p (r s) -> p r s", r=2
                    )
                    # cast2 lives on DVE only (see cast1 comment)
                    nc.vector.tensor_copy(cdst, csrc)
                # one output DMA per 4 iterations; Sync's HWDGE queue has
                # plenty of headroom and avoids GpSimd's SWDGE drain overhead
                nc.sync.dma_start(out[g], osb4[:])

    nc.finalize()
    return nc


def _get_program():
    if "nc" not in _STATE:
        _STATE["nc"] = _build_program()
        _STATE["consts"] = _constants()
        _STATE["sort_idx"] = _sort_idx()
    return _STATE["nc"]


def kernel(**inputs):
    global LAST_RESULT
    rgb = np.asarray(inputs["rgb_images_batch"], np.float32)
    assert rgb.shape == (N_CORES * B_PER_CORE, 3, 512, 512)
    B = N_CORES * B_PER_CORE
    xs = rgb.reshape(B, 3, NH, 32, 512).transpose(0, 2, 1, 3, 4)
    xs = (np.ascontiguousarray(xs).reshape(B, NH, 96, 512)
          - np.float32(0.5)).astype(np.float16)
    # group 4 block-rows with the partition dim outermost: [B, 4g, 96, 4r, 512]
    xs = np.ascontiguousarray(xs.reshape(B, NH // 4, 4, 96, 512).transpose(0, 1, 3, 2, 4))
    xs = xs.reshape(B, NH // 4, 96, 4, 512)
    nc = _get_program()
    w1, w2 = _STATE["consts"]
    sort_idx = _STATE["sort_idx"]

    in_maps = [
        {
            "x": xs[c * B_PER_CORE : (c + 1) * B_PER_CORE].reshape(GROUPS, 96, 4, 512),
            "w1": w1,
            "w2": w2,
        }
        for c in range(N_CORES)
    ]
    trace = os.environ.get("KERNEL_TRACE", "0") == "1"
    res = run_bass_kernel_spmd(
        nc, in_maps, core_ids=list(range(N_CORES)), trace=trace
    )
    LAST_RESULT = res

    outs = []
    for c in range(N_CORES):
        dev = res.results[c]["out"].astype(np.float32)  # [16, 128, 1536]
        dev = dev.reshape(GROUPS, 128, 4, 384).transpose(0, 2, 1, 3)
        dev = dev.reshape(ITERS, 128, 384)
        # [it=(b,br), p=(gxl,u), col=(kk, c, v)]
        a = dev.reshape(B_PER_CORE, NH, 4, 32, 4, 3, 32)  # b,br,gxl,u,kk,c,v
        a = a.transpose(0, 5, 6, 3, 1, 4, 2)  # b,c,v,u,br,kk,gxl
        a = np.ascontiguousarray(a).reshape(B_PER_CORE, 3, 1024, NH, NH)
        a = a[:, :, sort_idx, :, :]
        outs.append(a.reshape(B_PER_CORE, 3 * 1024, NH, NH))
    return np.concatenate(outs, axis=0)


# revision 15
# speedup vs baseline: 1.2188x; 1.2188x over previous
"""Trainium2 Bass kernel for DCTEncoderLayer — v3.2 "stationary swap, batched DMA".

Separable 32x32 DCT with the YCbCr mix folded into stage-1 weights.
Stage 1 runs with the IMAGE as the PE stationary operand:

    t1T[x, (c,v)] = img_chunk[(c',y), x].T @ W1[(c',y), (c,v)]

which lands the stage-1 result already transposed (x on partitions) —
no DVE stream-transpose is needed anywhere.  Stage 2 is a single
128-partition block-diagonal DCT along x':

    out[(gxl,u), (kk,c,v)] = W2bd[(gxl,x'), (gxl,u)].T @ t1s[(gxl,x'), (kk,c,v)]

v3.2 vs v3.1: DMAs carry ~350ns fixed cost each, so input DMAs batch 4
block-rows (host lays x out so each partition reads 4KB contiguously)
and output DMAs batch 4 iterations' results; stage-2 PSUM pairs two
iterations per 2-bank tile so each cast2 covers 768 columns.  Output
DMAs issue from the otherwise idle GpSimd DGE queue, input from Sync.

Per block-row iteration (64 per core, grouped by 4):
  - 4 stage-1 matmuls: stationary img chunk [96,128] fp16, moving W1 [96,96]
  - cast1 (ACT/DVE alt): t1T PSUM f32 [128, 4x(96 of 128)] -> SBUF fp16 [128,384]
  - 1 stage-2 matmul: stationary W2bd [128,128] fp16, moving [128,384] fp16
  - per 2 iters: cast2 (ACT/DVE alt) o2p PSUM f32 -> fp16 into the group's
    staging tile; per 4 iters: one output DMA [128, 1536] fp16.
Host upcasts, permutes axes and applies the frequency sort.
"""

import os
import sys

try:
    import concourse.bass  # noqa: F401
except ImportError:
    sys.path.insert(0, "/opt/trn_rl_repo")

import numpy as np

import concourse.bacc as bacc
import concourse.bass as bass
import concourse.mybir as mybir
import concourse.tile as tile
from concourse.bass_utils import run_bass_kernel_spmd

F32 = mybir.dt.float32
F16 = mybir.dt.float16

BS = 32
N_CORES = 8
B_PER_CORE = 4
NH = 16
ITERS = B_PER_CORE * NH  # 64 block-rows per core
GROUPS = ITERS // 4      # 16 groups of 4 block-rows

_STATE = {}
LAST_RESULT = None


def _dct_mat():
    y = np.arange(BS)
    v = np.arange(BS)[:, None]
    c = np.cos((2 * y + 1) * v * np.pi / (2 * BS))
    c[0, :] *= 1.0 / np.sqrt(2.0)
    return c / 4.0


def _sort_idx():
    mag = np.zeros((BS, BS), dtype=np.float64)
    for v in range(BS):
        for u in range(BS):
            mag[v, u] = np.linalg.norm(np.array([v, u], dtype=np.int64))
    return np.argsort(mag.reshape(-1))


def _constants():
    cs = _dct_mat()
    a2 = np.array(
        [
            [2 * 0.299, 2 * 0.587, 2 * 0.114],
            [2 * 0.564 * -0.299, 2 * 0.564 * -0.587, 2 * 0.564 * (1 - 0.114)],
            [2 * 0.713 * (1 - 0.299), 2 * 0.713 * -0.587, 2 * 0.713 * -0.114],
        ],
        np.float64,
    )
    w1 = np.zeros((96, 96))  # [(c', y), (c, v)]
    for cp in range(3):
        for c in range(3):
            w1[cp * 32 : (cp + 1) * 32, c * 32 : (c + 1) * 32] = a2[c, cp] * cs.T
    w2 = np.zeros((128, 128))  # [(gxl, x'), (gxl, u)] block diagonal over gxl
    for g in range(4):
        w2[g * 32 : (g + 1) * 32, g * 32 : (g + 1) * 32] = cs.T
    return w1.astype(np.float16), w2.astype(np.float16)


def _build_program():
    nc = bacc.Bacc(trn_type="TRN2")
    # host pre-groups 4 block-rows so each partition's 4KB is contiguous
    x = nc.dram_tensor("x", [GROUPS, 96, 4, 512], F16, kind="ExternalInput")
    w1 = nc.dram_tensor("w1", [96, 96], F16, kind="ExternalInput")
    w2 = nc.dram_tensor("w2", [128, 128], F16, kind="ExternalInput")
    out = nc.dram_tensor("out", [GROUPS, 128, 1536], F16, kind="ExternalOutput")

    with tile.TileContext(nc) as tc:
        with (
            tc.tile_pool(name="const", bufs=1) as constp,
            tc.tile_pool(name="pin", bufs=5) as pin,
            tc.tile_pool(name="pmid", bufs=8) as pmid,
            tc.tile_pool(name="pout", bufs=4) as pout,
            tc.tile_pool(name="psA", bufs=4, space="PSUM") as psA,
            tc.tile_pool(name="psB", bufs=2, space="PSUM") as psB,
        ):
            w1s = constp.tile([96, 96], F16)
            w2s = constp.tile([128, 128], F16)
            nc.sync.dma_start(w1s[:], w1[:])
            nc.sync.dma_start(w2s[:], w2[:])

            for g in range(GROUPS):
                img4 = pin.tile([96, 2048], F16, tag="img4")
                nc.sync.dma_start(
                    img4[:].rearrange("p (r x) -> p r x", r=4), x[g]
                )
                osb4 = pout.tile([128, 1536], F16, tag="osb4")
                o2p = None
                for j in range(4):
                    # stage 1 (stationary swap): 4 chunk slots, one PSUM bank
                    t1T = psA.tile([128, 512], F32, tag="t1T")
                    for k in range(4):
                        nc.tensor.matmul(
                            t1T[:, k * 128 : k * 128 + 96],
                            img4[:, j * 512 + k * 128 : j * 512 + (k + 1) * 128],
                            w1s[:],
                            start=True,
                            stop=True,
                        )
                    # cast1 (ACT only): pack 4 chunk slots -> fp16 [128, 384].
                    # keeping cast1 off DVE avoids queueing it behind cast2,
                    # which would serialize the PE->cast1->PE->cast2 chain
                    t1s = pmid.tile([128, 384], F16, tag="t1s")
                    src = t1T[:].rearrange("p (k s) -> p k s", k=4)[:, :, 0:96]
                    dst = t1s[:].rearrange("p (k s) -> p k s", k=4)
                    nc.scalar.copy(dst, src)
                    # stage 2: one matmul; two iterations share a 2-bank tile
                    if j % 2 == 0:
                        o2p = psB.tile([128, 1024], F32, tag="o2p")
                    nc.tensor.matmul(
                        o2p[:, (j % 2) * 512 : (j % 2) * 512 + 384],
                        w2s[:],
                        t1s[:],
                        start=True,
                        stop=True,
                    )
                    # cast2 (DVE only) covers both halves once the pair is done
                    if j % 2 == 1:
                        csrc = o2p[:].rearrange("p (r s) -> p r s", r=2)[:, :, 0:384]
                        cdst = osb4[:, (j - 1) * 384 : (j + 1) * 384].rearrange(
                            "p (r s) -> p r s", r=2
                        )
                        nc.vector.tensor_copy(cdst, csrc)
                # one output DMA per 4 iterations, on the GpSimd DGE queue
                nc.gpsimd.dma_start(out[g], osb4[:])

    nc.finalize()
    return nc


def _get_program():
    if "nc" not in _STATE:
        _STATE["nc"] = _build_program()
        _STATE["consts"] = _constants()
        _STATE["sort_idx"] = _sort_idx()
    return _STATE["nc"]


def kernel(**inputs):
    global LAST_RESULT
    rgb = np.asarray(inputs["rgb_images_batch"], np.float32)
    assert rgb.shape == (N_CORES * B_PER_CORE, 3, 512, 512)
    B = N_CORES * B_PER_CORE
    xs = rgb.reshape(B, 3, NH, 32, 512).transpose(0, 2, 1, 3, 4)
    xs = (np.ascontiguousarray(xs).reshape(B, NH, 96, 512)
          - np.float32(0.5)).astype(np.float16)
    # group 4 block-rows with the partition dim outermost: [B, 4g, 96, 4r, 512]
    xs = np.ascontiguousarray(xs.reshape(B, NH // 4, 4, 96, 512).transpose(0, 1, 3, 2, 4))
    xs = xs.reshape(B, NH // 4, 96, 4, 512)
    nc = _get_program()
    w1, w2 = _STATE["consts"]
    sort_idx = _STATE["sort_idx"]

    in_maps = [
        {
            "x": xs[c * B_PER_CORE : (c + 1) * B_PER_CORE].reshape(GROUPS, 96, 4, 512),
            "w1": w1,
            "w2": w2,
        }
        for c in range(N_CORES)
    ]
    trace = os.environ.get("KERNEL_TRACE", "0") == "1"
    res = run_bass_kernel_spmd(
        nc, in_maps, core_ids=list(range(N_CORES)), trace=trace
    )
    LAST_RESULT = res

    outs = []
    for c in range(N_CORES):
        dev = res.results[c]["out"].astype(np.float32)  # [16, 128, 1536]
        dev = dev.reshape(GROUPS, 128, 4, 384).transpose(0, 2, 1, 3)
        dev = dev.reshape(ITERS, 128, 384)
        # [it=(b,br), p=(gxl,u), col=(kk, c, v)]
        a = dev.reshape(B_PER_CORE, NH, 4, 32, 4, 3, 32)  # b,br,gxl,u,kk,c,v
        a = a.transpose(0, 5, 6, 3, 1, 4, 2)  # b,c,v,u,br,kk,gxl
        a = np.ascontiguousarray(a).reshape(B_PER_CORE, 3, 1024, NH, NH)
        a = a[:, :, sort_idx, :, :]
        outs.append(a.reshape(B_PER_CORE, 3 * 1024, NH, NH))
    return np.concatenate(outs, axis=0)
